# revision 1
# baseline (speedup 1.0000x reference)
"""Trainium2 Bass kernel for sparse graph attention (edge softmax + scatter-sum).

Strategy: dst-shard edges across 8 cores (each core owns a balanced set of
destination nodes and all edges pointing at them), so segment ops are core-local
and no collectives are needed. Within a core, nodes are packed into blocks of
<=128 nodes whose total in-degree fits CAP edges; each block's edges are laid
out contiguously (padded with slot=-1 dummies). On device, per 128-edge tile:

  - one-hot matmuls on TensorE implement the query gather (q[dst[e]]) and the
    weighted scatter-sum (feat[n] += ex[e]*v[e], denom[n] += ex[e])
  - logits come from a mask matmul that reduces k*q products over the 16-dim
    head groups; exp runs on ScalarE; softmax uses the max-free identity
    exp(l)/sum(exp(l)) == exp(l-m)/sum(exp(l-m))
  - final normalization feat/denom happens per node block.
"""

import math

import numpy as np

import concourse.bass as bass  # noqa: F401  (bass types reachable via bacc)
import concourse.tile as tile
from concourse import bacc, mybir
from concourse.bass_utils import run_bass_kernel_spmd

F32 = mybir.dt.float32
AOT = mybir.AluOpType
AFT = mybir.ActivationFunctionType

N_CORES = 8
TILE = 128          # edges per tile (PE contraction dim)
CAP = 2048          # edge capacity per node block
T = CAP // TILE     # 16 edge-tiles per block
GRP = 4             # tiles per group (512-col matmul moving operand)
NG = T // GRP
F = 128             # H*DK feature dim
FV = 256            # H*C*M value dim
H = 8
SCALE = 1.0 / math.sqrt(128.0)

# engine choice for elementwise stages (flip for perf experiments)
PROD_ENGINE = "gpsimd"   # k*q elementwise product
S4_ENGINE = "gpsimd"     # scatter one-hot build


def _pack_nodes(edge_dst: np.ndarray, n_nodes: int):
    """Assign nodes to cores (degree-balanced snake) and pack each core's nodes
    into blocks of <=TILE nodes and <=CAP total in-degree."""
    deg = np.bincount(edge_dst, minlength=n_nodes).astype(np.int64)
    order = np.argsort(-deg, kind="stable")
    pat = np.concatenate([np.arange(N_CORES), np.arange(N_CORES - 1, -1, -1)])
    reps = (n_nodes + 2 * N_CORES - 1) // (2 * N_CORES)
    core_of_rank = np.tile(pat, reps)[:n_nodes]
    node_core = np.empty(n_nodes, np.int32)
    node_core[order] = core_of_rank.astype(np.int32)

    node_block = np.empty(n_nodes, np.int32)
    node_slot = np.empty(n_nodes, np.int32)
    n_blocks = 0
    for c in range(N_CORES):
        nodes_c = order[core_of_rank == c]  # desc-degree order
        nz = len(nodes_c)
        zz = np.empty(nz, np.int64)
        zz[0::2] = np.arange((nz + 1) // 2)
        zz[1::2] = nz - 1 - np.arange(nz // 2)
        nodes_c = nodes_c[zz]  # zigzag big/small interleave
        d = deg[nodes_c]
        blk = np.empty(nz, np.int32)
        slot = np.empty(nz, np.int32)
        b = 0
        esum = 0
        cnt = 0
        for i in range(nz):
            dd = int(d[i])
            if esum + dd > CAP or cnt == TILE:
                b += 1
                esum = 0
                cnt = 0
            blk[i] = b
            slot[i] = cnt
            esum += dd
            cnt += 1
        node_block[nodes_c] = blk
        node_slot[nodes_c] = slot
        n_blocks = max(n_blocks, b + 1)
    return node_core, node_block, node_slot, n_blocks


def _build_program(n_blocks: int, e_pad: int, nq: int):
    """Build + compile the per-core Bass program (same on all 8 cores)."""
    nc = bacc.Bacc("TRN2", target_bir_lowering=False, debug=False,
                   enable_asserts=True, num_devices=N_CORES)

    kt = nc.dram_tensor("kt", [F, e_pad], F32, kind="ExternalInput")
    v = nc.dram_tensor("v", [e_pad, FV], F32, kind="ExternalInput")
    q = nc.dram_tensor("q", [nq, F], F32, kind="ExternalInput")
    slot = nc.dram_tensor("slot", [1, e_pad], F32, kind="ExternalInput")
    slott = nc.dram_tensor("slott", [F, n_blocks * T], F32, kind="ExternalInput")
    iota = nc.dram_tensor("iota", [F, TILE], F32, kind="ExternalInput")
    iotap = nc.dram_tensor("iotap", [F, 1], F32, kind="ExternalInput")
    hm = nc.dram_tensor("hm", [F, H], F32, kind="ExternalInput")

    feat = nc.dram_tensor("feat", [nq, FV], F32, kind="ExternalOutput")
    plog = nc.dram_tensor("plog", [e_pad, H], F32, kind="ExternalOutput")

    with tile.TileContext(nc) as tc:
        with (
            tc.tile_pool(name="const", bufs=1) as pc,
            tc.tile_pool(name="blk", bufs=2) as pb,
            tc.tile_pool(name="grp", bufs=3) as pg,
            tc.tile_pool(name="out", bufs=2) as po,
            tc.tile_pool(name="ps", bufs=2, space="PSUM") as pp,
        ):
            iota_t = pc.tile([F, TILE], F32)
            nc.sync.dma_start(iota_t[:], iota.ap()[:])
            iotap_t = pc.tile([F, 1], F32)
            nc.sync.dma_start(iotap_t[:], iotap.ap()[:])
            hm_t = pc.tile([F, H], F32)
            nc.sync.dma_start(hm_t[:], hm.ap()[:])
            slott_t = pc.tile([F, n_blocks * T], F32)
            nc.sync.dma_start(slott_t[:], slott.ap()[:])
            ones_t = pc.tile([1, F], F32)
            nc.vector.memset(ones_t[:], 1.0)

            for b in range(n_blocks):
                qb = pb.tile([TILE, F], F32)
                nc.sync.dma_start(qb[:], q.ap()[b * TILE:(b + 1) * TILE, :])
                srow = pb.tile([1, CAP], F32)
                nc.sync.dma_start(srow[:], slot.ap()[:, b * CAP:(b + 1) * CAP])
                ktb = pb.tile([F, CAP], F32)
                nc.sync.dma_start(ktb[:], kt.ap()[:, b * CAP:(b + 1) * CAP])
                vb = pb.tile([TILE, T, FV], F32)
                nc.sync.dma_start(
                    vb[:],
                    v.ap()[b * CAP:(b + 1) * CAP, :].rearrange(
                        "(t p) c -> p t c", p=TILE),
                )
                plogb = pb.tile([TILE, T, H], F32)
                featx = pp.tile([TILE, FV + H], F32)

                for g in range(NG):
                    e0 = g * GRP * TILE  # edge offset within block
                    # broadcast slot values to all partitions via K=1 matmul
                    slotb_ps = pp.tile([F, GRP * TILE], F32)
                    nc.tensor.matmul(
                        slotb_ps[:], ones_t[:],
                        srow[:, e0:e0 + GRP * TILE])
                    slotb = pg.tile([F, GRP * TILE], F32)
                    nc.scalar.copy(slotb[:], slotb_ps[:])
                    # gather one-hot ST[n, e] = (slot[e] == n)
                    st = pg.tile([F, GRP * TILE], F32)
                    nc.vector.tensor_scalar(
                        st[:], slotb[:], iotap_t[:], None, AOT.is_equal)
                    # gathered queries, transposed: qg[f, e] = q[slot[e], f]
                    qg_ps = pp.tile([F, GRP * TILE], F32)
                    nc.tensor.matmul(qg_ps[:], qb[:], st[:])
                    qg = pg.tile([F, GRP * TILE], F32)
                    nc.scalar.copy(qg[:], qg_ps[:])
                    # prod[f, e] = k[f, e] * qg[f, e]
                    prod = pg.tile([F, GRP * TILE], F32)
                    prod_eng = nc.gpsimd if PROD_ENGINE == "gpsimd" else nc.vector
                    prod_eng.tensor_tensor(
                        prod[:], ktb[:, e0:e0 + GRP * TILE], qg[:], AOT.mult)
                    # scatter one-hot S[e, n] = (n == slot[e]) per tile
                    s4 = pg.tile([TILE, GRP * TILE], F32)
                    s4_eng = nc.gpsimd if S4_ENGINE == "gpsimd" else nc.vector
                    for t in range(GRP):
                        col = b * T + g * GRP + t
                        s4_eng.tensor_scalar(
                            s4[:, t * TILE:(t + 1) * TILE], iota_t[:],
                            slott_t[:, col:col + 1], None, AOT.is_equal)
                    # logits[e, h] = sum_f prod[f, e] * hm[f, h]
                    lg_ps = pp.tile([TILE, GRP, H], F32)
                    for t in range(GRP):
                        nc.tensor.matmul(
                            lg_ps[:, t, :],
                            prod[:, t * TILE:(t + 1) * TILE], hm_t[:])
                    # prelogits out tile (scaled)
                    nc.scalar.mul(
                        plogb[:, g * GRP:(g + 1) * GRP, :], lg_ps[:], SCALE)
                    # yx[:, t, 0:256] = v * ex ; yx[:, t, 256:264] = ex
                    yx = pg.tile([TILE, GRP, FV + H], F32)
                    nc.scalar.activation(
                        yx[:, :, FV:FV + H], lg_ps[:], AFT.Exp, scale=SCALE)
                    ex_b = (yx[:, :, FV:FV + H]
                            .unsqueeze(3).broadcast_to([TILE, GRP, H, 32]))
                    nc.vector.tensor_tensor(
                        yx[:, :, 0:FV].rearrange("p t (h j) -> p t h j", j=32),
                        vb[:, g * GRP:(g + 1) * GRP, :].rearrange(
                            "p t (h j) -> p t h j", j=32),
                        ex_b, AOT.mult)
                    # scatter-accumulate into node block accumulators
                    for t in range(GRP):
                        first = (g == 0 and t == 0)
                        last = (g == NG - 1 and t == GRP - 1)
                        nc.tensor.matmul(
                            featx[:], s4[:, t * TILE:(t + 1) * TILE],
                            yx[:, t, :], start=first, stop=last)

                # epilogue: out[n] = feat[n] / (denom[n] + 1e-9)
                den = po.tile([TILE, H], F32)
                nc.vector.tensor_scalar(
                    den[:], featx[:, FV:FV + H], 1e-9, None, AOT.add)
                rec = po.tile([TILE, H], F32)
                nc.vector.reciprocal(rec[:], den[:])
                ob = po.tile([TILE, FV], F32)
                nc.vector.tensor_tensor(
                    ob[:].rearrange("p (h j) -> p h j", j=32),
                    featx[:, 0:FV].rearrange("p (h j) -> p h j", j=32),
                    rec[:].unsqueeze(2).broadcast_to([TILE, H, 32]),
                    AOT.mult)
                nc.sync.dma_start(feat.ap()[b * TILE:(b + 1) * TILE, :], ob[:])
                nc.sync.dma_start(
                    plog.ap()[b * CAP:(b + 1) * CAP, :].rearrange(
                        "(t p) h -> p t h", p=TILE),
                    plogb[:])

    nc.compile()
    return nc


def _prep(key_e, query_n, value_e, edge_dst):
    """Host-side sharding: returns in_maps + reassembly info."""
    n_nodes = query_n.shape[0]
    n_edges = key_e.shape[0]
    k2 = np.ascontiguousarray(key_e.reshape(n_edges, F))
    q2 = np.ascontiguousarray(query_n.reshape(n_nodes, F))
    v2 = np.ascontiguousarray(value_e.reshape(n_edges, FV))

    node_core, node_block, node_slot, n_blocks = _pack_nodes(edge_dst, n_nodes)
    e_pad = n_blocks * CAP
    nq = n_blocks * TILE

    iota = np.broadcast_to(np.arange(TILE, dtype=np.float32), (F, TILE)).copy()
    iotap = np.arange(F, dtype=np.float32).reshape(F, 1).copy()
    hmm = np.zeros((F, H), np.float32)
    hmm[np.arange(F), np.arange(F) // 16] = 1.0

    e_core = node_core[edge_dst]
    e_blk = node_block[edge_dst]
    e_slot = node_slot[edge_dst]

    in_maps = []
    asm = []  # per-core (edge_ids, edge_pos, nodes_c, node_pos)
    for c in range(N_CORES):
        ids = np.flatnonzero(e_core == c)
        blk = e_blk[ids]
        srt = np.argsort(blk, kind="stable")
        ids = ids[srt]
        blk = blk[srt]
        cnts = np.bincount(blk, minlength=n_blocks)
        assert cnts.max() <= CAP, f"block overflow on core {c}: {cnts.max()}"
        ofs = np.concatenate([[0], np.cumsum(cnts)[:-1]])
        rank = np.arange(len(ids)) - np.repeat(ofs, cnts)
        pos = blk.astype(np.int64) * CAP + rank

        ktc = np.zeros((F, e_pad), np.float32)
        ktc[:, pos] = k2[ids].T
        vc = np.zeros((e_pad, FV), np.float32)
        vc[pos] = v2[ids]
        slot_c = np.full((1, e_pad), -1.0, np.float32)
        slot_c[0, pos] = e_slot[ids].astype(np.float32)
        slott_c = np.ascontiguousarray(
            slot_c.reshape(n_blocks * T, TILE).T)
        qc = np.zeros((nq, F), np.float32)
        nodes_c = np.flatnonzero(node_core == c)
        node_pos = node_block[nodes_c].astype(np.int64) * TILE + node_slot[nodes_c]
        qc[node_pos] = q2[nodes_c]

        in_maps.append({
            "kt": ktc, "v": vc, "q": qc, "slot": slot_c, "slott": slott_c,
            "iota": iota, "iotap": iotap, "hm": hmm,
        })
        asm.append((ids, pos, nodes_c, node_pos))
    return in_maps, asm, n_blocks, e_pad, nq


_PROGRAM_CACHE = {}


def kernel(key_e, query_n, value_e, edge_dst):
    key_e = np.asarray(key_e, dtype=np.float32)
    query_n = np.asarray(query_n, dtype=np.float32)
    value_e = np.asarray(value_e, dtype=np.float32)
    edge_dst = np.asarray(edge_dst, dtype=np.int32)

    n_nodes = query_n.shape[0]
    n_edges = key_e.shape[0]
    h, c_, m_ = value_e.shape[1], value_e.shape[2], value_e.shape[3]

    in_maps, asm, n_blocks, e_pad, nq = _prep(key_e, query_n, value_e, edge_dst)

    ck = (n_blocks, e_pad, nq)
    if ck not in _PROGRAM_CACHE:
        _PROGRAM_CACHE[ck] = _build_program(*ck)
    nc = _PROGRAM_CACHE[ck]

    res = run_bass_kernel_spmd(nc, in_maps, list(range(N_CORES)), trace=False)

    out = np.empty((n_nodes, FV), np.float32)
    plog_full = np.empty((n_edges, H), np.float32)
    for c in range(N_CORES):
        ids, pos, nodes_c, node_pos = asm[c]
        out[nodes_c] = res.results[c]["feat"][node_pos]
        plog_full[ids] = res.results[c]["plog"][pos]
    return out.reshape(n_nodes, h * c_, m_), plog_full
